# revision 25
# baseline (speedup 1.0000x reference)
"""Clustered Linformer Attention — TRN2 Bass kernel, batch-parallel over 8 NeuronCores.

v2: bf16 upstream + host-side layout.
Per core (one batch element b):
  Host:  xT = x_b^T (bf16), wq/wk/wv bf16, E/F bf16 pre-shuffled to per-(h,g)
         contiguous DMA blocks, y returned transposed (host un-transposes).
  A:  q^T = wq^T-blocks @ xT-slices ; k,v = xT-blocks^T @ wk/wv   (PE, bf16)
  B:  kp/vp = k/v-blocks^T E/F, col-tiled: the two 64-wide heads of a d-block
      run concurrently in the PE array (tile_position from out base_partition),
      separate PSUM banks per parity; DVE accumulates across n-groups.
  C:  scores^T_h = kp_h^T q_h^T, row-tiled head pairs (K=64); exp via one ACT
      call per head over a 2-bank [128,2,512] PSUM tile (fused 1/sqrt(d)).
  F:  out_raw^T_h = vp2_h @ expT (ones-column extracts softmax row-sums)
  N:  reciprocal + PE broadcast matmul + GpSimd multiply
  G:  y^T = wd^T-blocks @ concat^T (bias per-partition), written bf16
"""
import sys
import numpy as np

for _p in ("/opt/trn_rl_repo", "/root/.axon_site/_ro/trn_rl_repo"):
    if _p not in sys.path:
        sys.path.insert(0, _p)

import ml_dtypes
import concourse.bacc as bacc
import concourse.tile as tile
from concourse import mybir
from concourse.bass_utils import run_bass_kernel_spmd

B, N, D = 8, 4096, 512
H, R = 8, 256
DEP = D // H          # 64
P = 128
NG = 8                # n-groups for phase A/B
GN = N // NG          # 512 rows per group
NS = 8                # n-strips for phase C..G
SN = N // NS          # 512 cols per strip
F32 = mybir.dt.float32
F32R = mybir.dt.float32r
BF16 = mybir.dt.bfloat16
EXPF = mybir.ActivationFunctionType.Exp
BF = ml_dtypes.bfloat16

_cache = {}


def build_program(repeat=1, variant="full"):
    key = ("nc", repeat, variant)
    if key in _cache:
        return _cache[key]
    nc = bacc.Bacc("TRN2", target_bir_lowering=False, debug=False)
    xT = nc.dram_tensor("xT", [D, N], BF16, kind="ExternalInput").ap()
    wqkv = nc.dram_tensor("wqkv", [D, 3, D], BF16, kind="ExternalInput").ap()
    wd = nc.dram_tensor("wd", [D, D], F32, kind="ExternalInput").ap()
    EF = nc.dram_tensor("EF", [H, NG, P, 2, 4, R], BF16, kind="ExternalInput").ap()
    ident_in = nc.dram_tensor("ident", [P, P], F32, kind="ExternalInput").ap()
    hb_in = nc.dram_tensor("hb", [P, D], F32, kind="ExternalInput").ap()
    ones_in = nc.dram_tensor("ones", [P, 1], BF16, kind="ExternalInput").ap()
    bT_in = nc.dram_tensor("bT", [P, 4], F32, kind="ExternalInput").ap()
    y = nc.dram_tensor("y", [D, N], BF16, kind="ExternalOutput").ap()

    with tile.TileContext(nc) as tc, nc.allow_low_precision(reason="bf16 kernel"):
      for _rep in range(repeat):
        with tc.tile_pool(name="outer", bufs=1) as po:
            # ---- persistent tiles ----
            qT = [po.tile([P, N], BF16, tag=f"qT{c}", name=f"qT{c}") for c in range(4)]
            kpA = [po.tile([P, R], F32, tag=f"kpA{p}", name=f"kpA{p}") for p in range(4)]
            vpA = [po.tile([P, R], F32, tag=f"vpA{p}", name=f"vpA{p}") for p in range(4)]
            kpS = [po.tile([P, R], BF16, tag=f"kpS{p}", name=f"kpS{p}") for p in range(4)]
            vp2 = [[po.tile([P, P], BF16, tag=f"vp2_{h}_{rc}", name=f"vp2_{h}_{rc}")
                    for rc in range(2)] for h in range(H)]
            bT_t = po.tile([P, 4], F32, tag="bT", name="bT")
            wd_t = [po.tile([P, D], BF16, tag=f"wd{c}", name=f"wd{c}") for c in range(4)]
            ident = po.tile([P, P], F32, tag="ident", name="ident")
            hbr = [po.tile([P, P], BF16, tag=f"hb{p}", name=f"hb{p}") for p in range(4)]
            ones_f = po.tile([P, 1], BF16, tag="ones", name="ones")
            S_t = [po.tile([P, SN], F32, tag=f"S{i}", name=f"S{i}")
                   for i in range(2)]
            Sr_t = [po.tile([P, SN], BF16, tag=f"Sr{i}", name=f"Sr{i}")
                    for i in range(2)]

            nc.sync.dma_start(bT_t[:], bT_in)
            nc.sync.dma_start(ones_f[:], ones_in)
            # rows 8.. of S never written per-strip; keep them finite (the
            # full-tile reciprocal then keeps Sr rows 8.. finite too)
            nc.gpsimd.memset(S_t[0][:], 1.0)
            nc.gpsimd.memset(S_t[1][:], 1.0)

            # ================= PHASE A+B =================
            with tc.tile_pool(name="pw", bufs=1) as pw, \
                 tc.tile_pool(name="pxs", bufs=6) as pxs, \
                 tc.tile_pool(name="pkv", bufs=8) as pkv, \
                 tc.tile_pool(name="pef", bufs=7) as pef, \
                 tc.tile_pool(name="psA", bufs=3, space="PSUM") as psA, \
                 tc.tile_pool(name="psB", bufs=1, space="PSUM") as psB:


                # critical-path first: x-slice + QKV weights gate first MMs
                xs0 = pxs.tile([P, 4, GN], BF16, tag="xs", name="xs")
                nc.sync.dma_start(
                    xs0[:], xT[:, 0:GN].rearrange("(c p) n -> p c n", p=P))
                wqkv_t = [pw.tile([P, 3, D], BF16, tag=f"wqkv{c}", name=f"wqkv{c}")
                          for c in range(4)]
                for c in range(4):
                    nc.sync.dma_start(wqkv_t[c][:], wqkv[c * P:(c + 1) * P])
                # constants that need rounding to f32r (stationary use)
                stage = pw.tile([P, D], F32, tag="hbstage", name="hbstage")
                nc.sync.dma_start(stage[:], hb_in)
                for p in range(4):
                    nc.vector.tensor_copy(hbr[p][:], stage[:, p * P:(p + 1) * P])
                wds = pw.tile([P, 4, D], F32, tag="wdraw", name="wdraw")
                nc.sync.dma_start(wds[:], wd.rearrange("(c p) d -> p c d", p=P))
                for c in range(4):
                    nc.vector.tensor_copy(wd_t[c][:], wds[:, c, :])
                nc.sync.dma_start(ident[:], ident_in)
                xs_next = None
                for g in range(NG):
                    n0 = g * GN
                    # per-group column slices of xT (moving for q, stationary
                    # for k/v) — all four 128-row blocks in one DMA
                    if g == 0:
                        xs_all = xs0
                    elif g % 2 == 1:
                        xs_all = xs_next
                    else:
                        xs_all = pxs.tile([P, 4, GN], BF16, tag="xs", name="xs")
                        nc.sync.dma_start(
                            xs_all[:],
                            xT[:, n0:n0 + GN].rearrange("(c p) n -> p c n", p=P))

                    # q^T — group-paired: both groups' MMs share each wq
                    # stationary back-to-back
                    if g % 2 == 0:
                        xs_next = pxs.tile([P, 4, GN], BF16, tag="xs", name="xs")
                        nc.sync.dma_start(
                            xs_next[:],
                            xT[:, n0 + GN:n0 + 2 * GN].rearrange(
                                "(c p) n -> p c n", p=P))
                        for dq in range(4):
                            qp0 = psA.tile([P, GN], F32, tag="qkv", name="qkv")
                            qp1 = psA.tile([P, GN], F32, tag="qkv", name="qkv")
                            for c in range(4):
                                nc.tensor.matmul(
                                    qp0[:], wqkv_t[c][:, 0, dq * P:(dq + 1) * P],
                                    xs_all[:, c, :],
                                    start=(c == 0), stop=(c == 3))
                                nc.tensor.matmul(
                                    qp1[:], wqkv_t[c][:, 0, dq * P:(dq + 1) * P],
                                    xs_next[:, c, :],
                                    start=(c == 0), stop=(c == 3))
                            nc.scalar.copy(qT[dq][:, n0:n0 + GN], qp0[:])
                            nc.scalar.copy(qT[dq][:, n0 + GN:n0 + 2 * GN], qp1[:])
                    # k, v (n-major, bf16 for B-phase stationaries); k and v
                    # MMs interleaved per c so consecutive MMs share the same
                    # stationary (one weight load serves both)
                    kg = [pkv.tile([P, D], BF16, tag="kg", name="kg") for i in range(4)]
                    vg = [pkv.tile([P, D], BF16, tag="vg", name="vg") for i in range(4)]
                    for i in range(4):
                        kp_ = psA.tile([P, D], F32, tag="qkv", name="qkv")
                        vp_ = psA.tile([P, D], F32, tag="qkv", name="qkv")
                        for c in range(4):
                            nc.tensor.matmul(
                                kp_[:], xs_all[:, c, i * P:(i + 1) * P],
                                wqkv_t[c][:, 1, :],
                                start=(c == 0), stop=(c == 3))
                            nc.tensor.matmul(
                                vp_[:], xs_all[:, c, i * P:(i + 1) * P],
                                wqkv_t[c][:, 2, :],
                                start=(c == 0), stop=(c == 3))
                        nc.scalar.copy(kg[i][:], kp_[:])
                        nc.scalar.copy(vg[i][:], vp_[:])
                    # B: project k, v through E_h, F_h. Column-tiled: the two
                    # 64-wide heads of a d-block run concurrently in the PE
                    # array (par0 -> array cols 0-63, par1 -> 64-127), each
                    # accumulating into its own PSUM bank.
                    for pidx in range(4):
                        kpg = [psB.tile([P, SN], F32, tag=f"kpg{par}", name=f"kpg{par}")
                               for par in range(2)]
                        vpg = [psB.tile([P, SN], F32, tag=f"vpg{par}", name=f"vpg{par}")
                               for par in range(2)]
                        gsrc = 0 if variant == "noef" else g
                        EFh = []
                        for par in range(2):
                            h = 2 * pidx + par
                            t = pef.tile([P, 2, 4, R], BF16, tag="ef", name="ef")
                            nc.sync.dma_start(t[:], EF[h, gsrc])
                            EFh.append(t)
                        # interleave parities: alternating 64-wide column
                        # groups, separate PSUM banks per parity
                        for i in range(4):
                            for par in range(2):
                                ro = DEP * par
                                cs = slice(pidx * P + ro, pidx * P + ro + DEP)
                                nc.tensor.matmul(
                                    kpg[par][ro:ro + DEP, 0:R],
                                    kg[i][:, cs], EFh[par][:, 0, i, :],
                                    start=(i == 0), stop=(i == 3))
                        for i in range(4):
                            for par in range(2):
                                ro = DEP * par
                                cs = slice(pidx * P + ro, pidx * P + ro + DEP)
                                nc.tensor.matmul(
                                    vpg[par][ro:ro + DEP, 0:R],
                                    vg[i][:, cs], EFh[par][:, 1, i, :],
                                    start=(i == 0), stop=(i == 3))
                        for par in range(2):
                            sl = slice(DEP * par, DEP * par + DEP)
                            if g == 0:
                                nc.vector.tensor_copy(kpA[pidx][sl, :], kpg[par][sl, 0:R])
                                nc.vector.tensor_copy(vpA[pidx][sl, :], vpg[par][sl, 0:R])
                            else:
                                nc.vector.tensor_add(
                                    kpA[pidx][sl, :], kpA[pidx][sl, :], kpg[par][sl, 0:R])
                                nc.vector.tensor_add(
                                    vpA[pidx][sl, :], vpA[pidx][sl, :], vpg[par][sl, 0:R])

                # round kp to bf16 stationaries; transpose vp into natural
                # layout with ones-column row-sum extractor
                for p in range(4):
                    nc.vector.tensor_copy(kpS[p][:], kpA[p][:])
                    for rc in range(2):
                        vt = psA.tile([P, P], F32, tag="qkv", name="qkv")
                        nc.tensor.transpose(
                            vt[:], vpA[p][:, rc * P:(rc + 1) * P], ident[:])
                        for par in range(2):
                            h = 2 * p + par
                            ro = DEP * par
                            oro = DEP * (1 - par)
                            nc.vector.tensor_copy(
                                vp2[h][rc][:, ro:ro + DEP], vt[:, ro:ro + DEP])
                            nc.vector.tensor_copy(
                                vp2[h][rc][:, oro:oro + 1], ones_f[:])
                            nc.gpsimd.memset(vp2[h][rc][:, oro + 1:oro + DEP], 0.0)

            # ================= PHASE C..G =================
            if variant in ("ab_only", "ab_flat"):
                dummy = po.tile([P, N], BF16, tag="dummy", name="dummy")
                nc.vector.tensor_copy(dummy[:, 0:N], qT[0][:])
                for dq in range(4):
                    nc.sync.dma_start(y[dq * P:(dq + 1) * P, :], dummy[:])
                continue
            with tc.tile_pool(name="pexp", bufs=4) as pexp, \
                 tc.tile_pool(name="pstag", bufs=2) as pstag, \
                 tc.tile_pool(name="pcs", bufs=2) as pcs, \
                 tc.tile_pool(name="pbc", bufs=3) as pbc, \
                 tc.tile_pool(name="psml", bufs=2) as psml, \
                 tc.tile_pool(name="psS", bufs=2, space="PSUM") as psS, \
                 tc.tile_pool(name="psF", bufs=2, space="PSUM") as psF, \
                 tc.tile_pool(name="psY", bufs=2, space="PSUM") as psY:
                # strips processed in pairs: fop / broadcast / dense MMs for
                # the two strips are emitted back-to-back per stationary so
                # consecutive MMs reuse the loaded weights
                for sp in range(NS // 2):
                    ss = (2 * sp, 2 * sp + 1)
                    csR2 = [pcs.tile([P, 4, SN], BF16, tag="csR", name="csR")
                            for _ in range(2)]
                    stg2 = [pstag.tile([P, H, SN], F32, tag="stag", name="stag")
                            for _ in range(2)]
                    for c in range(4):  # head pair (2c, 2c+1), row-tiled K=64
                        expT2 = []
                        for si, s in enumerate(ss):
                            c0 = s * SN
                            scp = [psS.tile([P, 2, SN], F32, tag="sc", name="sc")
                                   for par in range(2)]
                            for rc in range(2):
                                for par in range(2):
                                    rs = slice(DEP * par, DEP * par + DEP)
                                    nc.tensor.matmul(
                                        scp[par][:, rc, :],
                                        kpS[c][rs, rc * P:(rc + 1) * P],
                                        qT[c][rs, c0:c0 + SN],
                                        start=True, stop=True)
                            expT = [pexp.tile([P, 2, SN], BF16, tag="expT",
                                              name="expT") for par in range(2)]
                            for par in range(2):
                                nc.scalar.activation(
                                    expT[par][:], scp[par][:], EXPF,
                                    scale=float(1.0 / np.sqrt(np.float32(DEP))))
                            expT2.append(expT)
                        for par in range(2):
                            h = 2 * c + par
                            fp2 = [psF.tile([P, SN], F32, tag="fo", name="fo")
                                   for _ in range(2)]
                            for rc in range(2):
                                for si in range(2):
                                    nc.tensor.matmul(
                                        fp2[si][:], vp2[h][rc][:],
                                        expT2[si][par][:, rc, :],
                                        start=(rc == 0), stop=(rc == 1))
                            for si in range(2):
                                nc.vector.tensor_copy(stg2[si][:, h, :], fp2[si][:])
                    # sums: even heads leave their row-sum at partition 64,
                    # odd heads at partition 0; two strided gathers fill
                    # S rows [0:4] (even heads) and [4:8] (odd heads)
                    for si, s in enumerate(ss):
                        nc.sync.dma_start(S_t[si][0:4, :], stg2[si][64:65, 0:8:2, :])
                        nc.sync.dma_start(S_t[si][4:8, :], stg2[si][0:1, 1:8:2, :])
                        nc.vector.reciprocal(Sr_t[si][:], S_t[si][:])
                    bcs2 = [[None, None] for _ in range(4)]
                    for p in range(4):
                        bcp2 = [psY.tile([P, SN], F32, tag="y", name="y")
                                for _ in range(2)]
                        for si in range(2):
                            nc.tensor.matmul(bcp2[si][:], hbr[p][:],
                                             Sr_t[si][:], start=True, stop=True)
                        for si in range(2):
                            bcs = pbc.tile([P, SN], F32, tag="bcs", name="bcs")
                            nc.vector.tensor_copy(bcs[:], bcp2[si][:])
                            bcs2[p][si] = bcs
                        for par in range(2):
                            h = 2 * p + par
                            ro = DEP * par
                            for si in range(2):
                                nc.gpsimd.tensor_mul(
                                    csR2[si][ro:ro + DEP, p, :],
                                    stg2[si][ro:ro + DEP, h, :],
                                    bcs2[p][si][ro:ro + DEP, :])
                    # y^T = wd^T @ concat^T  (bias is per-partition here)
                    ys2 = [psml.tile([P, 4, SN], BF16, tag="ysb", name="ysb")
                           for _ in range(2)]
                    for dq in range(4):
                        yp2 = [psY.tile([P, SN], F32, tag="y", name="y")
                               for _ in range(2)]
                        for c2 in range(4):
                            for si in range(2):
                                nc.tensor.matmul(
                                    yp2[si][:], wd_t[c2][:, dq * P:(dq + 1) * P],
                                    csR2[si][:, c2, :],
                                    start=(c2 == 0), stop=(c2 == 3))
                        for si in range(2):
                            nc.vector.tensor_scalar_add(
                                ys2[si][:, dq, :], yp2[si][:], bT_t[:, dq:dq + 1])
                    for si, s in enumerate(ss):
                        c0 = s * SN
                        nc.sync.dma_start(
                            y[:, c0:c0 + SN].rearrange("(dq p) n -> p dq n", p=P),
                            ys2[si][:])

    nc.compile()
    _cache[key] = nc
    return nc


def make_in_maps(x, wq, wk, wv, E, F, w_dense, b_dense):
    x = np.asarray(x, dtype=np.float32)
    E = np.asarray(E, np.float32)
    F = np.asarray(F, np.float32)
    # per-(head, group) contiguous DMA blocks: [H, NG, P, 4, R]
    esh = lambda t: np.ascontiguousarray(
        t.reshape(H, NG, 4, P, R).transpose(0, 1, 3, 2, 4).astype(BF))
    wqkv = np.stack([np.asarray(w, np.float32).astype(BF)
                     for w in (wq, wk, wv)], axis=1)
    consts = {
        "wqkv": np.ascontiguousarray(wqkv),
        "wd": np.ascontiguousarray(np.asarray(w_dense, np.float32)),
        "EF": np.ascontiguousarray(np.stack([esh(E), esh(F)], axis=3)),
        "ident": np.eye(P, dtype=np.float32),
        "hb": _make_hb(),
        "ones": np.ones((P, 1), dtype=BF),
        "bT": np.ascontiguousarray(
            np.asarray(b_dense, np.float32).reshape(4, P).T),
    }
    return [{"xT": np.ascontiguousarray(x[b].T.astype(BF)), **consts}
            for b in range(B)]


def _make_hb():
    # S/Sr row order: even heads h at row h//2, odd heads at 4 + h//2
    hb = np.zeros((P, D), dtype=np.float32)
    for p in range(4):
        for m in range(P):
            par = m // DEP
            hb[p + 4 * par, p * P + m] = 1.0
    return hb


def kernel(x, wq, wk, wv, E, F, w_dense, b_dense):
    nc = build_program()
    in_maps = make_in_maps(x, wq, wk, wv, E, F, w_dense, b_dense)
    res = run_bass_kernel_spmd(nc, in_maps, list(range(B)))
    out = np.stack(
        [np.asarray(res.results[b]["y"]).astype(np.float32).T for b in range(B)],
        axis=0)
    return np.ascontiguousarray(out)


def postprocess(y_stack):
    """bench2 helper: [B, D, N] bf16 -> [B, N, D] f32."""
    return np.ascontiguousarray(
        np.asarray(y_stack).astype(np.float32).transpose(0, 2, 1))


# revision 27
# speedup vs baseline: 1.0097x; 1.0097x over previous
"""Clustered Linformer Attention — TRN2 Bass kernel, batch-parallel over 8 NeuronCores.

v2: bf16 upstream + host-side layout.
Per core (one batch element b):
  Host:  xT = x_b^T (bf16), wq/wk/wv bf16, E/F bf16 pre-shuffled to per-(h,g)
         contiguous DMA blocks, y returned transposed (host un-transposes).
  A:  q^T = wq^T-blocks @ xT-slices ; k,v = xT-blocks^T @ wk/wv   (PE, bf16)
  B:  kp/vp = k/v-blocks^T E/F, col-tiled: the two 64-wide heads of a d-block
      run concurrently in the PE array (tile_position from out base_partition),
      separate PSUM banks per parity; DVE accumulates across n-groups.
  C:  scores^T_h = kp_h^T q_h^T, row-tiled head pairs (K=64); exp via one ACT
      call per head over a 2-bank [128,2,512] PSUM tile (fused 1/sqrt(d)).
  F:  out_raw^T_h = vp2_h @ expT (ones-column extracts softmax row-sums)
  N:  reciprocal + PE broadcast matmul + GpSimd multiply
  G:  y^T = wd^T-blocks @ concat^T (bias per-partition), written bf16
"""
import sys
import numpy as np

for _p in ("/opt/trn_rl_repo", "/root/.axon_site/_ro/trn_rl_repo"):
    if _p not in sys.path:
        sys.path.insert(0, _p)

import ml_dtypes
import concourse.bacc as bacc
import concourse.tile as tile
from concourse import mybir
from concourse.bass_utils import run_bass_kernel_spmd

B, N, D = 8, 4096, 512
H, R = 8, 256
DEP = D // H          # 64
P = 128
NG = 8                # n-groups for phase A/B
GN = N // NG          # 512 rows per group
NS = 8                # n-strips for phase C..G
SN = N // NS          # 512 cols per strip
F32 = mybir.dt.float32
F32R = mybir.dt.float32r
BF16 = mybir.dt.bfloat16
EXPF = mybir.ActivationFunctionType.Exp
BF = ml_dtypes.bfloat16

_cache = {}


def build_program(repeat=1, variant="full"):
    key = ("nc", repeat, variant)
    if key in _cache:
        return _cache[key]
    nc = bacc.Bacc("TRN2", target_bir_lowering=False, debug=False)
    xT = nc.dram_tensor("xT", [D, N], BF16, kind="ExternalInput").ap()
    wqkv = nc.dram_tensor("wqkv", [D, 3, D], BF16, kind="ExternalInput").ap()
    wd = nc.dram_tensor("wd", [D, D], F32, kind="ExternalInput").ap()
    EF = nc.dram_tensor("EF", [H, NG, P, 2, 4, R], BF16, kind="ExternalInput").ap()
    ident_in = nc.dram_tensor("ident", [P, P], F32, kind="ExternalInput").ap()
    hb_in = nc.dram_tensor("hb", [P, D], F32, kind="ExternalInput").ap()
    ones_in = nc.dram_tensor("ones", [P, 1], BF16, kind="ExternalInput").ap()
    bT_in = nc.dram_tensor("bT", [P, 4], F32, kind="ExternalInput").ap()
    y = nc.dram_tensor("y", [D, N], BF16, kind="ExternalOutput").ap()

    with tile.TileContext(nc) as tc, nc.allow_low_precision(reason="bf16 kernel"):
      for _rep in range(repeat):
        with tc.tile_pool(name="outer", bufs=1) as po:
            # ---- persistent tiles ----
            qT = [po.tile([P, N], BF16, tag=f"qT{c}", name=f"qT{c}") for c in range(4)]
            kpA = [po.tile([P, R], F32, tag=f"kpA{p}", name=f"kpA{p}") for p in range(4)]
            vpA = [po.tile([P, R], F32, tag=f"vpA{p}", name=f"vpA{p}") for p in range(4)]
            kpS = [po.tile([P, R], BF16, tag=f"kpS{p}", name=f"kpS{p}") for p in range(4)]
            vp2 = [[po.tile([P, P], BF16, tag=f"vp2_{h}_{rc}", name=f"vp2_{h}_{rc}")
                    for rc in range(2)] for h in range(H)]
            bT_t = po.tile([P, 4], F32, tag="bT", name="bT")
            wd_t = [po.tile([P, D], BF16, tag=f"wd{c}", name=f"wd{c}") for c in range(4)]
            ident = po.tile([P, P], F32, tag="ident", name="ident")
            hbr = [po.tile([P, P], BF16, tag=f"hb{p}", name=f"hb{p}") for p in range(4)]
            ones_f = po.tile([P, 1], BF16, tag="ones", name="ones")
            S_t = [po.tile([P, SN], F32, tag=f"S{i}", name=f"S{i}")
                   for i in range(2)]
            Sr_t = [po.tile([P, SN], BF16, tag=f"Sr{i}", name=f"Sr{i}")
                    for i in range(2)]

            nc.sync.dma_start(bT_t[:], bT_in)
            nc.sync.dma_start(ones_f[:], ones_in)
            # rows 8.. of S never written per-strip; keep them finite (the
            # full-tile reciprocal then keeps Sr rows 8.. finite too)
            nc.gpsimd.memset(S_t[0][:], 1.0)
            nc.gpsimd.memset(S_t[1][:], 1.0)

            # ================= PHASE A+B =================
            with tc.tile_pool(name="pw", bufs=1) as pw, \
                 tc.tile_pool(name="pxs", bufs=6) as pxs, \
                 tc.tile_pool(name="pkv", bufs=8) as pkv, \
                 tc.tile_pool(name="pef", bufs=8) as pef, \
                 tc.tile_pool(name="psA", bufs=4, space="PSUM") as psA, \
                 tc.tile_pool(name="psB", bufs=1, space="PSUM") as psB:


                # critical-path first: x-slice + QKV weights gate first MMs
                xs0 = pxs.tile([P, 4, GN], BF16, tag="xs", name="xs")
                nc.sync.dma_start(
                    xs0[:], xT[:, 0:GN].rearrange("(c p) n -> p c n", p=P))
                wqkv_t = [pw.tile([P, 3, D], BF16, tag=f"wqkv{c}", name=f"wqkv{c}")
                          for c in range(4)]
                for c in range(4):
                    nc.sync.dma_start(wqkv_t[c][:], wqkv[c * P:(c + 1) * P])
                # constants that need rounding to f32r (stationary use)
                stage = pw.tile([P, D], F32, tag="hbstage", name="hbstage")
                nc.sync.dma_start(stage[:], hb_in)
                for p in range(4):
                    nc.vector.tensor_copy(hbr[p][:], stage[:, p * P:(p + 1) * P])
                wds = pw.tile([P, 4, D], F32, tag="wdraw", name="wdraw")
                nc.sync.dma_start(wds[:], wd.rearrange("(c p) d -> p c d", p=P))
                for c in range(4):
                    nc.vector.tensor_copy(wd_t[c][:], wds[:, c, :])
                nc.sync.dma_start(ident[:], ident_in)
                for g in range(NG):
                    n0 = g * GN
                    # per-group column slices of xT (moving for q, stationary
                    # for k/v) — all four 128-row blocks in one DMA
                    if g == 0:
                        xs_all = xs0
                    else:
                        xs_all = pxs.tile([P, 4, GN], BF16, tag="xs", name="xs")
                        nc.sync.dma_start(
                            xs_all[:],
                            xT[:, n0:n0 + GN].rearrange("(c p) n -> p c n", p=P))

                    # q^T
                    for dq in range(4):
                        qp = psA.tile([P, GN], F32, tag="qkv", name="qkv")
                        for c in range(4):
                            nc.tensor.matmul(
                                qp[:], wqkv_t[c][:, 0, dq * P:(dq + 1) * P],
                                xs_all[:, c, :],
                                start=(c == 0), stop=(c == 3))
                        nc.scalar.copy(qT[dq][:, n0:n0 + GN], qp[:])
                    # k, v (n-major, bf16 for B-phase stationaries); k and v
                    # MMs interleaved per c so consecutive MMs share the same
                    # stationary (one weight load serves both)
                    kg = [pkv.tile([P, D], BF16, tag="kg", name="kg") for i in range(4)]
                    vg = [pkv.tile([P, D], BF16, tag="vg", name="vg") for i in range(4)]
                    for i in range(4):
                        kp_ = psA.tile([P, D], F32, tag="qkv", name="qkv")
                        vp_ = psA.tile([P, D], F32, tag="qkv", name="qkv")
                        for c in range(4):
                            nc.tensor.matmul(
                                kp_[:], xs_all[:, c, i * P:(i + 1) * P],
                                wqkv_t[c][:, 1, :],
                                start=(c == 0), stop=(c == 3))
                            nc.tensor.matmul(
                                vp_[:], xs_all[:, c, i * P:(i + 1) * P],
                                wqkv_t[c][:, 2, :],
                                start=(c == 0), stop=(c == 3))
                        nc.scalar.copy(kg[i][:], kp_[:])
                        nc.scalar.copy(vg[i][:], vp_[:])
                    # B: project k, v through E_h, F_h. Column-tiled: the two
                    # 64-wide heads of a d-block run concurrently in the PE
                    # array (par0 -> array cols 0-63, par1 -> 64-127), each
                    # accumulating into its own PSUM bank.
                    for pidx in range(4):
                        kpg = [psB.tile([P, SN], F32, tag=f"kpg{par}", name=f"kpg{par}")
                               for par in range(2)]
                        vpg = [psB.tile([P, SN], F32, tag=f"vpg{par}", name=f"vpg{par}")
                               for par in range(2)]
                        gsrc = 0 if variant == "noef" else g
                        EFh = []
                        for par in range(2):
                            h = 2 * pidx + par
                            t = pef.tile([P, 2, 4, R], BF16, tag="ef", name="ef")
                            nc.sync.dma_start(t[:], EF[h, gsrc])
                            EFh.append(t)
                        # interleave parities: alternating 64-wide column
                        # groups, separate PSUM banks per parity
                        for i in range(4):
                            for par in range(2):
                                ro = DEP * par
                                cs = slice(pidx * P + ro, pidx * P + ro + DEP)
                                nc.tensor.matmul(
                                    kpg[par][ro:ro + DEP, 0:R],
                                    kg[i][:, cs], EFh[par][:, 0, i, :],
                                    start=(i == 0), stop=(i == 3))
                        for i in range(4):
                            for par in range(2):
                                ro = DEP * par
                                cs = slice(pidx * P + ro, pidx * P + ro + DEP)
                                nc.tensor.matmul(
                                    vpg[par][ro:ro + DEP, 0:R],
                                    vg[i][:, cs], EFh[par][:, 1, i, :],
                                    start=(i == 0), stop=(i == 3))
                        for par in range(2):
                            sl = slice(DEP * par, DEP * par + DEP)
                            if g == 0:
                                nc.vector.tensor_copy(kpA[pidx][sl, :], kpg[par][sl, 0:R])
                                nc.vector.tensor_copy(vpA[pidx][sl, :], vpg[par][sl, 0:R])
                            else:
                                nc.vector.tensor_add(
                                    kpA[pidx][sl, :], kpA[pidx][sl, :], kpg[par][sl, 0:R])
                                nc.vector.tensor_add(
                                    vpA[pidx][sl, :], vpA[pidx][sl, :], vpg[par][sl, 0:R])

                # round kp to bf16 stationaries; transpose vp into natural
                # layout with ones-column row-sum extractor
                for p in range(4):
                    nc.vector.tensor_copy(kpS[p][:], kpA[p][:])
                    for rc in range(2):
                        vt = psA.tile([P, P], F32, tag="qkv", name="qkv")
                        nc.tensor.transpose(
                            vt[:], vpA[p][:, rc * P:(rc + 1) * P], ident[:])
                        for par in range(2):
                            h = 2 * p + par
                            ro = DEP * par
                            oro = DEP * (1 - par)
                            nc.vector.tensor_copy(
                                vp2[h][rc][:, ro:ro + DEP], vt[:, ro:ro + DEP])
                            nc.vector.tensor_copy(
                                vp2[h][rc][:, oro:oro + 1], ones_f[:])
                            nc.gpsimd.memset(vp2[h][rc][:, oro + 1:oro + DEP], 0.0)

            # ================= PHASE C..G =================
            if variant in ("ab_only", "ab_flat"):
                dummy = po.tile([P, N], BF16, tag="dummy", name="dummy")
                nc.vector.tensor_copy(dummy[:, 0:N], qT[0][:])
                for dq in range(4):
                    nc.sync.dma_start(y[dq * P:(dq + 1) * P, :], dummy[:])
                continue
            with tc.tile_pool(name="pexp", bufs=4) as pexp, \
                 tc.tile_pool(name="pstag", bufs=2) as pstag, \
                 tc.tile_pool(name="pcs", bufs=2) as pcs, \
                 tc.tile_pool(name="pbc", bufs=2) as pbc, \
                 tc.tile_pool(name="psml", bufs=2) as psml, \
                 tc.tile_pool(name="psS", bufs=2, space="PSUM") as psS, \
                 tc.tile_pool(name="psF", bufs=2, space="PSUM") as psF, \
                 tc.tile_pool(name="psY", bufs=2, space="PSUM") as psY:
                for s in range(NS):
                    c0 = s * SN
                    csR = pcs.tile([P, 4, SN], BF16, tag="csR", name="csR")
                    stg = pstag.tile([P, H, SN], F32, tag="stag", name="stag")
                    for c in range(4):  # head pair (2c, 2c+1), row-tiled K=64
                        scp = [psS.tile([P, 2, SN], F32, tag="sc", name="sc")
                               for par in range(2)]
                        for rc in range(2):
                            for par in range(2):
                                rs = slice(DEP * par, DEP * par + DEP)
                                nc.tensor.matmul(
                                    scp[par][:, rc, :],
                                    kpS[c][rs, rc * P:(rc + 1) * P],
                                    qT[c][rs, c0:c0 + SN],
                                    start=True, stop=True)
                        expT = [pexp.tile([P, 2, SN], BF16, tag="expT", name="expT")
                                for par in range(2)]
                        for par in range(2):
                            nc.scalar.activation(
                                expT[par][:], scp[par][:], EXPF,
                                scale=float(1.0 / np.sqrt(np.float32(DEP))))
                        for par in range(2):
                            h = 2 * c + par
                            fp = psF.tile([P, SN], F32, tag="fo", name="fo")
                            for rc in range(2):
                                nc.tensor.matmul(
                                    fp[:], vp2[h][rc][:], expT[par][:, rc, :],
                                    start=(rc == 0), stop=(rc == 1))
                            nc.vector.tensor_copy(stg[:, h, :], fp[:])
                    # sums: even heads leave their row-sum at partition 64,
                    # odd heads at partition 0; two strided gathers fill
                    # S rows [0:4] (even heads) and [4:8] (odd heads)
                    nc.sync.dma_start(S_t[s % 2][0:4, :], stg[64:65, 0:8:2, :])
                    nc.sync.dma_start(S_t[s % 2][4:8, :], stg[0:1, 1:8:2, :])
                    nc.vector.reciprocal(Sr_t[s % 2][:], S_t[s % 2][:])
                    for p in range(4):
                        bcp = psY.tile([P, SN], F32, tag="y", name="y")
                        nc.tensor.matmul(bcp[:], hbr[p][:],
                                         Sr_t[s % 2][:], start=True, stop=True)
                        bcs = pbc.tile([P, SN], F32, tag="bcs", name="bcs")
                        nc.scalar.copy(bcs[:], bcp[:])
                        for par in range(2):
                            h = 2 * p + par
                            ro = DEP * par
                            nc.gpsimd.tensor_mul(
                                csR[ro:ro + DEP, p, :],
                                stg[ro:ro + DEP, h, :],
                                bcs[ro:ro + DEP, :])
                    # y^T = wd^T @ concat^T  (bias is per-partition in this layout)
                    ys = psml.tile([P, 4, SN], BF16, tag="ysb", name="ysb")
                    for dq in range(4):
                        yp = psY.tile([P, SN], F32, tag="y", name="y")
                        for c2 in range(4):
                            nc.tensor.matmul(
                                yp[:], wd_t[c2][:, dq * P:(dq + 1) * P],
                                csR[:, c2, :],
                                start=(c2 == 0), stop=(c2 == 3))
                        nc.vector.tensor_scalar_add(
                            ys[:, dq, :], yp[:], bT_t[:, dq:dq + 1])
                    nc.sync.dma_start(
                        y[:, c0:c0 + SN].rearrange("(dq p) n -> p dq n", p=P),
                        ys[:])

    nc.compile()
    _cache[key] = nc
    return nc


def make_in_maps(x, wq, wk, wv, E, F, w_dense, b_dense):
    x = np.asarray(x, dtype=np.float32)
    E = np.asarray(E, np.float32)
    F = np.asarray(F, np.float32)
    # per-(head, group) contiguous DMA blocks: [H, NG, P, 4, R]
    esh = lambda t: np.ascontiguousarray(
        t.reshape(H, NG, 4, P, R).transpose(0, 1, 3, 2, 4).astype(BF))
    wqkv = np.stack([np.asarray(w, np.float32).astype(BF)
                     for w in (wq, wk, wv)], axis=1)
    consts = {
        "wqkv": np.ascontiguousarray(wqkv),
        "wd": np.ascontiguousarray(np.asarray(w_dense, np.float32)),
        "EF": np.ascontiguousarray(np.stack([esh(E), esh(F)], axis=3)),
        "ident": np.eye(P, dtype=np.float32),
        "hb": _make_hb(),
        "ones": np.ones((P, 1), dtype=BF),
        "bT": np.ascontiguousarray(
            np.asarray(b_dense, np.float32).reshape(4, P).T),
    }
    return [{"xT": np.ascontiguousarray(x[b].T.astype(BF)), **consts}
            for b in range(B)]


def _make_hb():
    # S/Sr row order: even heads h at row h//2, odd heads at 4 + h//2
    hb = np.zeros((P, D), dtype=np.float32)
    for p in range(4):
        for m in range(P):
            par = m // DEP
            hb[p + 4 * par, p * P + m] = 1.0
    return hb


def kernel(x, wq, wk, wv, E, F, w_dense, b_dense):
    nc = build_program()
    in_maps = make_in_maps(x, wq, wk, wv, E, F, w_dense, b_dense)
    res = run_bass_kernel_spmd(nc, in_maps, list(range(B)))
    out = np.stack(
        [np.asarray(res.results[b]["y"]).astype(np.float32).T for b in range(B)],
        axis=0)
    return np.ascontiguousarray(out)


def postprocess(y_stack):
    """bench2 helper: [B, D, N] bf16 -> [B, N, D] f32."""
    return np.ascontiguousarray(
        np.asarray(y_stack).astype(np.float32).transpose(0, 2, 1))


# revision 28
# speedup vs baseline: 1.0880x; 1.0775x over previous
"""Clustered Linformer Attention — TRN2 Bass kernel, batch-parallel over 8 NeuronCores.

v2: bf16 upstream + host-side layout.
Per core (one batch element b):
  Host:  xT = x_b^T (bf16), wq/wk/wv bf16, E/F bf16 pre-shuffled to per-(h,g)
         contiguous DMA blocks, y returned transposed (host un-transposes).
  A:  q^T = wq^T-blocks @ xT-slices ; k,v = xT-blocks^T @ wk/wv   (PE, bf16)
  B:  kp/vp = k/v-blocks^T E/F, col-tiled: the two 64-wide heads of a d-block
      run concurrently in the PE array (tile_position from out base_partition),
      separate PSUM banks per parity; DVE accumulates across n-groups.
  C:  scores^T_h = kp_h^T q_h^T, row-tiled head pairs (K=64); exp via one ACT
      call per head over a 2-bank [128,2,512] PSUM tile (fused 1/sqrt(d)).
  F:  out_raw^T_h = vp2_h @ expT (ones-column extracts softmax row-sums)
  N:  reciprocal + PE broadcast matmul + GpSimd multiply
  G:  y^T = wd^T-blocks @ concat^T (bias per-partition), written bf16
"""
import sys
import numpy as np

for _p in ("/opt/trn_rl_repo", "/root/.axon_site/_ro/trn_rl_repo"):
    if _p not in sys.path:
        sys.path.insert(0, _p)

import ml_dtypes
import concourse.bacc as bacc
import concourse.tile as tile
from concourse import mybir
from concourse.bass_utils import run_bass_kernel_spmd

B, N, D = 8, 4096, 512
H, R = 8, 256
DEP = D // H          # 64
P = 128
NG = 8                # n-groups for phase A/B
GN = N // NG          # 512 rows per group
NS = 8                # n-strips for phase C..G
SN = N // NS          # 512 cols per strip
F32 = mybir.dt.float32
F32R = mybir.dt.float32r
BF16 = mybir.dt.bfloat16
EXPF = mybir.ActivationFunctionType.Exp
BF = ml_dtypes.bfloat16

_cache = {}


def build_program(repeat=1, variant="full"):
    key = ("nc", repeat, variant)
    if key in _cache:
        return _cache[key]
    nc = bacc.Bacc("TRN2", target_bir_lowering=False, debug=False)
    xT = nc.dram_tensor("xT", [D, N], BF16, kind="ExternalInput").ap()
    wqkv = nc.dram_tensor("wqkv", [D, 3, D], BF16, kind="ExternalInput").ap()
    wd = nc.dram_tensor("wd", [D, D], F32, kind="ExternalInput").ap()
    EF = nc.dram_tensor("EF", [H, NG, P, 2, 4, R], BF16, kind="ExternalInput").ap()
    ident_in = nc.dram_tensor("ident", [P, P], F32, kind="ExternalInput").ap()
    hb_in = nc.dram_tensor("hb", [P, D], F32, kind="ExternalInput").ap()
    ones_in = nc.dram_tensor("ones", [P, 1], BF16, kind="ExternalInput").ap()
    bT_in = nc.dram_tensor("bT", [P, 4], F32, kind="ExternalInput").ap()
    y = nc.dram_tensor("y", [D, N], BF16, kind="ExternalOutput").ap()

    with tile.TileContext(nc) as tc, nc.allow_low_precision(reason="bf16 kernel"):
      for _rep in range(repeat):
        with tc.tile_pool(name="outer", bufs=1) as po:
            # ---- persistent tiles ----
            qT = [po.tile([P, N], BF16, tag=f"qT{c}", name=f"qT{c}") for c in range(4)]
            kpA = [po.tile([P, R], F32, tag=f"kpA{p}", name=f"kpA{p}") for p in range(4)]
            vpA = [po.tile([P, R], F32, tag=f"vpA{p}", name=f"vpA{p}") for p in range(4)]
            kpS = [po.tile([P, R], BF16, tag=f"kpS{p}", name=f"kpS{p}") for p in range(4)]
            vp2 = [[po.tile([P, P], BF16, tag=f"vp2_{h}_{rc}", name=f"vp2_{h}_{rc}")
                    for rc in range(2)] for h in range(H)]
            bT_t = po.tile([P, 4], F32, tag="bT", name="bT")
            wd_t = [po.tile([P, D], BF16, tag=f"wd{c}", name=f"wd{c}") for c in range(4)]
            ident = po.tile([P, P], F32, tag="ident", name="ident")
            hbr = [po.tile([P, P], BF16, tag=f"hb{p}", name=f"hb{p}") for p in range(4)]
            ones_f = po.tile([P, 1], BF16, tag="ones", name="ones")
            S_t = [po.tile([P, SN], F32, tag=f"S{i}", name=f"S{i}")
                   for i in range(2)]
            Sr_t = [po.tile([P, SN], BF16, tag=f"Sr{i}", name=f"Sr{i}")
                    for i in range(2)]

            nc.sync.dma_start(bT_t[:], bT_in)
            nc.sync.dma_start(ones_f[:], ones_in)
            # rows 8.. of S never written per-strip; keep them finite (the
            # full-tile reciprocal then keeps Sr rows 8.. finite too)
            nc.gpsimd.memset(S_t[0][:], 1.0)
            nc.gpsimd.memset(S_t[1][:], 1.0)

            # ================= PHASE A+B =================
            with tc.tile_pool(name="pw", bufs=1) as pw, \
                 tc.tile_pool(name="pxs", bufs=6) as pxs, \
                 tc.tile_pool(name="pkv", bufs=8) as pkv, \
                 tc.tile_pool(name="pef", bufs=7) as pef, \
                 tc.tile_pool(name="psA", bufs=3, space="PSUM") as psA, \
                 tc.tile_pool(name="psB", bufs=1, space="PSUM") as psB:


                # critical-path first: x-slice + QKV weights gate first MMs
                xs0 = pxs.tile([P, 4, GN], BF16, tag="xs", name="xs")
                nc.sync.dma_start(
                    xs0[:], xT[:, 0:GN].rearrange("(c p) n -> p c n", p=P))
                wqkv_t = [pw.tile([P, 3, D], BF16, tag=f"wqkv{c}", name=f"wqkv{c}")
                          for c in range(4)]
                for c in range(4):
                    nc.sync.dma_start(wqkv_t[c][:], wqkv[c * P:(c + 1) * P])
                # constants that need rounding to f32r (stationary use)
                stage = pw.tile([P, D], F32, tag="hbstage", name="hbstage")
                nc.sync.dma_start(stage[:], hb_in)
                for p in range(4):
                    nc.vector.tensor_copy(hbr[p][:], stage[:, p * P:(p + 1) * P])
                wds = pw.tile([P, 4, D], F32, tag="wdraw", name="wdraw")
                nc.sync.dma_start(wds[:], wd.rearrange("(c p) d -> p c d", p=P))
                for c in range(4):
                    nc.vector.tensor_copy(wd_t[c][:], wds[:, c, :])
                nc.sync.dma_start(ident[:], ident_in)
                for g in range(NG):
                    n0 = g * GN
                    # per-group column slices of xT (moving for q, stationary
                    # for k/v) — all four 128-row blocks in one DMA
                    if g == 0:
                        xs_all = xs0
                    else:
                        xs_all = pxs.tile([P, 4, GN], BF16, tag="xs", name="xs")
                        nc.sync.dma_start(
                            xs_all[:],
                            xT[:, n0:n0 + GN].rearrange("(c p) n -> p c n", p=P))

                    # q^T
                    for dq in range(4):
                        qp = psA.tile([P, GN], F32, tag="qkv", name="qkv")
                        for c in range(4):
                            nc.tensor.matmul(
                                qp[:], wqkv_t[c][:, 0, dq * P:(dq + 1) * P],
                                xs_all[:, c, :],
                                start=(c == 0), stop=(c == 3))
                        nc.scalar.copy(qT[dq][:, n0:n0 + GN], qp[:])
                    # k, v (n-major, bf16 for B-phase stationaries); k and v
                    # MMs interleaved per c so consecutive MMs share the same
                    # stationary (one weight load serves both)
                    kg = [pkv.tile([P, D], BF16, tag="kg", name="kg") for i in range(4)]
                    vg = [pkv.tile([P, D], BF16, tag="vg", name="vg") for i in range(4)]
                    for i in range(4):
                        kp_ = psA.tile([P, D], F32, tag="qkv", name="qkv")
                        vp_ = psA.tile([P, D], F32, tag="qkv", name="qkv")
                        for c in range(4):
                            nc.tensor.matmul(
                                kp_[:], xs_all[:, c, i * P:(i + 1) * P],
                                wqkv_t[c][:, 1, :],
                                start=(c == 0), stop=(c == 3))
                            nc.tensor.matmul(
                                vp_[:], xs_all[:, c, i * P:(i + 1) * P],
                                wqkv_t[c][:, 2, :],
                                start=(c == 0), stop=(c == 3))
                        nc.scalar.copy(kg[i][:], kp_[:])
                        nc.scalar.copy(vg[i][:], vp_[:])
                    # B: project k, v through E_h, F_h. Column-tiled: the two
                    # 64-wide heads of a d-block run concurrently in the PE
                    # array (par0 -> array cols 0-63, par1 -> 64-127), each
                    # accumulating into its own PSUM bank.
                    for pidx in range(4):
                        kpg = [psB.tile([P, SN], F32, tag=f"kpg{par}", name=f"kpg{par}")
                               for par in range(2)]
                        vpg = [psB.tile([P, SN], F32, tag=f"vpg{par}", name=f"vpg{par}")
                               for par in range(2)]
                        gsrc = 0 if variant == "noef" else g
                        EFh = []
                        for par in range(2):
                            h = 2 * pidx + par
                            t = pef.tile([P, 2, 4, R], BF16, tag="ef", name="ef")
                            nc.sync.dma_start(t[:], EF[h, gsrc])
                            EFh.append(t)
                        # interleave parities: alternating 64-wide column
                        # groups, separate PSUM banks per parity
                        for i in range(4):
                            for par in range(2):
                                ro = DEP * par
                                cs = slice(pidx * P + ro, pidx * P + ro + DEP)
                                nc.tensor.matmul(
                                    kpg[par][ro:ro + DEP, 0:R],
                                    kg[i][:, cs], EFh[par][:, 0, i, :],
                                    start=(i == 0), stop=(i == 3))
                        for i in range(4):
                            for par in range(2):
                                ro = DEP * par
                                cs = slice(pidx * P + ro, pidx * P + ro + DEP)
                                nc.tensor.matmul(
                                    vpg[par][ro:ro + DEP, 0:R],
                                    vg[i][:, cs], EFh[par][:, 1, i, :],
                                    start=(i == 0), stop=(i == 3))
                        for par in range(2):
                            sl = slice(DEP * par, DEP * par + DEP)
                            if g == 0:
                                nc.vector.tensor_copy(kpA[pidx][sl, :], kpg[par][sl, 0:R])
                                nc.vector.tensor_copy(vpA[pidx][sl, :], vpg[par][sl, 0:R])
                            else:
                                nc.vector.tensor_add(
                                    kpA[pidx][sl, :], kpA[pidx][sl, :], kpg[par][sl, 0:R])
                                nc.vector.tensor_add(
                                    vpA[pidx][sl, :], vpA[pidx][sl, :], vpg[par][sl, 0:R])

                # round kp to bf16 stationaries; transpose vp into natural
                # layout with ones-column row-sum extractor
                for p in range(4):
                    nc.vector.tensor_copy(kpS[p][:], kpA[p][:])
                    for rc in range(2):
                        vt = psA.tile([P, P], F32, tag="qkv", name="qkv")
                        nc.tensor.transpose(
                            vt[:], vpA[p][:, rc * P:(rc + 1) * P], ident[:])
                        for par in range(2):
                            h = 2 * p + par
                            ro = DEP * par
                            oro = DEP * (1 - par)
                            nc.vector.tensor_copy(
                                vp2[h][rc][:, ro:ro + DEP], vt[:, ro:ro + DEP])
                            nc.vector.tensor_copy(
                                vp2[h][rc][:, oro:oro + 1], ones_f[:])
                            nc.gpsimd.memset(vp2[h][rc][:, oro + 1:oro + DEP], 0.0)

            # ================= PHASE C..G =================
            if variant in ("ab_only", "ab_flat"):
                dummy = po.tile([P, N], BF16, tag="dummy", name="dummy")
                nc.vector.tensor_copy(dummy[:, 0:N], qT[0][:])
                for dq in range(4):
                    nc.sync.dma_start(y[dq * P:(dq + 1) * P, :], dummy[:])
                continue
            with tc.tile_pool(name="pexp", bufs=4) as pexp, \
                 tc.tile_pool(name="pstag", bufs=2) as pstag, \
                 tc.tile_pool(name="pcs", bufs=2) as pcs, \
                 tc.tile_pool(name="pbc", bufs=2) as pbc, \
                 tc.tile_pool(name="psml", bufs=2) as psml, \
                 tc.tile_pool(name="psS", bufs=2, space="PSUM") as psS, \
                 tc.tile_pool(name="psF", bufs=2, space="PSUM") as psF, \
                 tc.tile_pool(name="psY", bufs=2, space="PSUM") as psY:
                for s in range(NS):
                    c0 = s * SN
                    csR = pcs.tile([P, 4, SN], BF16, tag="csR", name="csR")
                    stg = pstag.tile([P, H, SN], F32, tag="stag", name="stag")
                    for c in range(4):  # head pair (2c, 2c+1), row-tiled K=64
                        scp = [psS.tile([P, 2, SN], F32, tag="sc", name="sc")
                               for par in range(2)]
                        for rc in range(2):
                            for par in range(2):
                                rs = slice(DEP * par, DEP * par + DEP)
                                nc.tensor.matmul(
                                    scp[par][:, rc, :],
                                    kpS[c][rs, rc * P:(rc + 1) * P],
                                    qT[c][rs, c0:c0 + SN],
                                    start=True, stop=True)
                        expT = [pexp.tile([P, 2, SN], BF16, tag="expT", name="expT")
                                for par in range(2)]
                        for par in range(2):
                            nc.scalar.activation(
                                expT[par][:], scp[par][:], EXPF,
                                scale=float(1.0 / np.sqrt(np.float32(DEP))))
                        for par in range(2):
                            h = 2 * c + par
                            fp = psF.tile([P, SN], F32, tag="fo", name="fo")
                            for rc in range(2):
                                nc.tensor.matmul(
                                    fp[:], vp2[h][rc][:], expT[par][:, rc, :],
                                    start=(rc == 0), stop=(rc == 1))
                            nc.vector.tensor_copy(stg[:, h, :], fp[:])
                    # sums: even heads leave their row-sum at partition 64,
                    # odd heads at partition 0; two strided gathers fill
                    # S rows [0:4] (even heads) and [4:8] (odd heads)
                    nc.sync.dma_start(S_t[s % 2][0:4, :], stg[64:65, 0:8:2, :])
                    nc.sync.dma_start(S_t[s % 2][4:8, :], stg[0:1, 1:8:2, :])
                    nc.vector.reciprocal(Sr_t[s % 2][:], S_t[s % 2][:])
                    for p in range(4):
                        bcp = psY.tile([P, SN], F32, tag="y", name="y")
                        nc.tensor.matmul(bcp[:], hbr[p][:],
                                         Sr_t[s % 2][:], start=True, stop=True)
                        bcs = pbc.tile([P, SN], F32, tag="bcs", name="bcs")
                        nc.vector.tensor_copy(bcs[:], bcp[:])
                        for par in range(2):
                            h = 2 * p + par
                            ro = DEP * par
                            nc.gpsimd.tensor_mul(
                                csR[ro:ro + DEP, p, :],
                                stg[ro:ro + DEP, h, :],
                                bcs[ro:ro + DEP, :])
                    # y^T = wd^T @ concat^T  (bias is per-partition in this layout)
                    ys = psml.tile([P, 4, SN], BF16, tag="ysb", name="ysb")
                    for dq in range(4):
                        yp = psY.tile([P, SN], F32, tag="y", name="y")
                        for c2 in range(4):
                            nc.tensor.matmul(
                                yp[:], wd_t[c2][:, dq * P:(dq + 1) * P],
                                csR[:, c2, :],
                                start=(c2 == 0), stop=(c2 == 3))
                        nc.vector.tensor_scalar_add(
                            ys[:, dq, :], yp[:], bT_t[:, dq:dq + 1])
                    nc.sync.dma_start(
                        y[:, c0:c0 + SN].rearrange("(dq p) n -> p dq n", p=P),
                        ys[:])

    nc.compile()
    _cache[key] = nc
    return nc


def make_in_maps(x, wq, wk, wv, E, F, w_dense, b_dense):
    x = np.asarray(x, dtype=np.float32)
    E = np.asarray(E, np.float32)
    F = np.asarray(F, np.float32)
    # per-(head, group) contiguous DMA blocks: [H, NG, P, 4, R]
    esh = lambda t: np.ascontiguousarray(
        t.reshape(H, NG, 4, P, R).transpose(0, 1, 3, 2, 4).astype(BF))
    wqkv = np.stack([np.asarray(w, np.float32).astype(BF)
                     for w in (wq, wk, wv)], axis=1)
    consts = {
        "wqkv": np.ascontiguousarray(wqkv),
        "wd": np.ascontiguousarray(np.asarray(w_dense, np.float32)),
        "EF": np.ascontiguousarray(np.stack([esh(E), esh(F)], axis=3)),
        "ident": np.eye(P, dtype=np.float32),
        "hb": _make_hb(),
        "ones": np.ones((P, 1), dtype=BF),
        "bT": np.ascontiguousarray(
            np.asarray(b_dense, np.float32).reshape(4, P).T),
    }
    return [{"xT": np.ascontiguousarray(x[b].T.astype(BF)), **consts}
            for b in range(B)]


def _make_hb():
    # S/Sr row order: even heads h at row h//2, odd heads at 4 + h//2
    hb = np.zeros((P, D), dtype=np.float32)
    for p in range(4):
        for m in range(P):
            par = m // DEP
            hb[p + 4 * par, p * P + m] = 1.0
    return hb


def kernel(x, wq, wk, wv, E, F, w_dense, b_dense):
    nc = build_program()
    in_maps = make_in_maps(x, wq, wk, wv, E, F, w_dense, b_dense)
    res = run_bass_kernel_spmd(nc, in_maps, list(range(B)))
    out = np.stack(
        [np.asarray(res.results[b]["y"]).astype(np.float32).T for b in range(B)],
        axis=0)
    return np.ascontiguousarray(out)


def postprocess(y_stack):
    """bench2 helper: [B, D, N] bf16 -> [B, N, D] f32."""
    return np.ascontiguousarray(
        np.asarray(y_stack).astype(np.float32).transpose(0, 2, 1))
